# revision 97
# baseline (speedup 1.0000x reference)
"""VGCN encoder (2-layer GCN, shared normalized adjacency) on 8 Trainium2 cores.

Strategy: node-partitioned graph parallelism. Nodes are padded to
NPAD = 8*6272 and core c owns nodes [6272c, 6272(c+1)), split into 98 buckets
of 64. All edges (plus one self-edge per node, which realizes both GCN's +1
degree and the self-loop term) are routed to the core owning their dst node,
bucketed by dst bucket, and aggregated on-device with one-hot matmuls over
128-edge tiles (bf16 operands, fp32 psum):

    agg[bucket] += S.T @ us[src]   (S[e, j] = dst_local[e] == j, built on DVE)

Messages are fetched with SWDGE dma_gather (4 queues round-robin) from a
bf16 DRAM table of 256-byte rows ([HID bf16 payload | HID pad]) stored in a
(core, partition, bucket)-major permutation so bulk table writes are
contiguous DMAs. Each gather element is a 512-byte PAIR of adjacent rows
(idx = row//2 < 25088 fits int16 with no table split, and 512B descriptors
run ~20% faster per element than 256B ones); the even/odd row selection is
folded into two dead-marked one-hot S columns per 128-edge tile, so each
tile does two accumulating matmuls instead of one. dis = 1/sqrt(deg) is precomputed on the host and
FOLDED INTO x (x' = dis*x), so phase A is a pure matmul and its psum
drains through the otherwise-idle ACT engine as a bf16 cast. On real HW
the gathers are descriptor-rate-bound (~4 ns per 256B row); both layers'
row streams dominate the runtime, so everything else (S builds on DVE,
relu/dis^2 and the final dis scale as single ACT activations, phase-A
matmuls fed by two half-shard xT loads with a 4-deep psum pipeline) is
engineered to hide under the gather DMA.

Layer 1 input (x' @ W1, x' shipped in bf16) is computed REPLICATED on every
core for the whole graph - it is ~50 MFLOP, far cheaper than an AllGather
round - so the only collective is the AllGather of the layer-1 activations
(packed bf16, Shared-scratchpad output, expanded to padded rows locally;
measured far cheaper per-op than chunked/ReduceScatter alternatives).
Layer-2 aggregation runs feature-major (lhsT=msg) so both output projections
become 13 wide matmuls against [Wmu | Wlv] with one transpose per 128 nodes.

All host-prepared operands travel in ONE packed int16 input tensor (f32/bf16
sections bitcast on device) and both outputs leave in ONE tensor; together
with pipelined dispatch this keeps the per-execution dispatch overhead of the
axon tunnel (~0.8 ms flat per round trip) mostly off the measured
steady-state time.
"""

import sys

sys.path.insert(0, "/opt/trn_rl_repo")

import numpy as np

from concourse import bacc, mybir, tile
from concourse.bass_utils import run_bass_kernel_spmd
from concourse.masks import make_identity

F32 = mybir.dt.float32
BF16 = mybir.dt.bfloat16
I16 = mybir.dt.int16
I32 = mybir.dt.int32


class Cfg:
    def __init__(self, n=50000, e=800000, in_dim=128, hid=64, ncores=8,
                 shard_tiles=49, bw=64, half=32768, chunk_tiles=19, sbatch=16):
        self.N, self.E, self.IN, self.HID = n, e, in_dim, hid
        self.NCORES = ncores
        self.P = 128
        self.SHARD = shard_tiles * 128    # nodes per core
        self.NPAD = ncores * self.SHARD
        self.BW = bw                      # bucket width (psum partition dim)
        self.NBK = self.SHARD // bw       # buckets per core
        self.GBK = ncores * self.NBK      # global buckets
        self.HALF = half                  # gather-table split so int16 idx fit
        self.CH = chunk_tiles             # tiles (128 rows) per dma_gather
        self.SB = sbatch                  # tiles per batched one-hot build
        assert self.NPAD >= n and half <= 32768 and self.SHARD % bw == 0
        assert self.NBK <= 128 and self.NBK % 2 == 0


DEFAULT = Cfg()
MPOOL_BUFS = 6


def pack_offsets(cfg, nT):
    """Column offsets (int16 units) of each section in the packed input."""
    off, out = 0, {}
    for name, width in (("xT", cfg.NPAD), ("w1", 2 * cfg.HID),
                        ("wml", 4 * cfg.HID), ("dis", 2 * cfg.NBK),
                        ("disP", cfg.NBK), ("dl", 2 * nT),
                        ("idx", 8 * nT)):
        out[name] = off
        off += width
    out["W"] = off
    return out


def build_layout(edge_index, cfg=DEFAULT):
    """Static per-core edge streams plus the (identical-across-cores) tile
    structure. Table row of node n: c*SHARD + (r%BW)*NBK + r//BW, r=n%SHARD."""
    src = np.asarray(edge_index[0], np.int64)
    dst = np.asarray(edge_index[1], np.int64)
    NBK, BW = cfg.NBK, cfg.BW

    deg = np.bincount(dst, minlength=cfg.NPAD).astype(np.float64) + 1.0
    dis = (1.0 / np.sqrt(deg)).astype(np.float32)   # padding nodes: dis=1

    # Each gather element is a PAIR of adjacent 256B table rows (512B, the
    # sweet spot of the SWDGE descriptor rate); idx = row//2 < 25088 fits
    # int16 with no table-half split, and the even/odd row selection is
    # folded into two dead-marked one-hot S columns per tile.
    per_core = []
    cnts = np.zeros((cfg.NCORES, NBK), np.int64)
    for c in range(cfg.NCORES):
        m = (dst >= c * cfg.SHARD) & (dst < (c + 1) * cfg.SHARD)
        s = src[m]
        d = dst[m]
        selfn = np.arange(c * cfg.SHARD, (c + 1) * cfg.SHARD, dtype=np.int64)
        s = np.concatenate([s, selfn])
        d = np.concatenate([d, selfn])
        cc, rr = s // cfg.SHARD, s % cfg.SHARD
        row = cc * cfg.SHARD + (rr % BW) * NBK + rr // BW
        dr = d - c * cfg.SHARD
        key = dr // BW                   # dst bucket
        dl = dr % BW
        order = np.argsort(key, kind="stable")
        row, dl, key = row[order], dl[order], key[order]
        per_core.append((row, dl, key))
        cnts[c] = np.bincount(key, minlength=NBK)

    ntile = np.ceil(cnts.max(axis=0) / 128.0).astype(np.int64)
    nT = int(ntile.sum())
    tbs = np.repeat(np.arange(NBK), ntile)
    offs = np.concatenate([[0], np.cumsum(ntile)]) * 128

    cores = []
    for c in range(cfg.NCORES):
        row, dl, key = per_core[c]
        bounds = np.searchsorted(key, np.arange(NBK + 1))
        idx = np.zeros(nT * 128, np.int64)
        dlE = np.full(nT * 128, BW, np.int64)
        dlO = np.full(nT * 128, BW, np.int64)
        for b in range(NBK):
            lo, hi = bounds[b], bounds[b + 1]
            o = offs[b]
            idx[o:o + hi - lo] = row[lo:hi] >> 1
            par = row[lo:hi] & 1
            dlE[o:o + hi - lo] = np.where(par == 0, dl[lo:hi], BW)
            dlO[o:o + hi - lo] = np.where(par == 1, dl[lo:hi], BW)

        def wrap(stream):
            a = stream.reshape(-1, 16).T.astype(np.int16)
            return np.tile(a, (8, 1))   # replicated across the 8 q7 cores

        # dl columns interleaved: col 2t = even-row S, col 2t+1 = odd-row S
        dlEO = np.empty((128, 2 * nT), np.float32)
        dlEO[:, 0::2] = dlE.reshape(-1, 128).T
        dlEO[:, 1::2] = dlO.reshape(-1, 128).T

        # dis for own nodes in (partition=dl, bucket) layout
        dis_own = np.ascontiguousarray(
            dis[c * cfg.SHARD:(c + 1) * cfg.SHARD].reshape(NBK, BW).T)

        disP = np.ascontiguousarray(
            dis_own.reshape(BW, NBK // 2, 2).transpose(2, 0, 1)
            .reshape(2 * BW, NBK // 2))

        cores.append(dict(
            idx=wrap(idx), dl=dlEO,
            dis=dis_own, disP=disP,
        ))

    return dict(ntile=tuple(int(x) for x in ntile), tbs=tbs, nT=nT,
                cores=cores, dis_full=dis.astype(np.float32))


def build_program(layout, cfg=DEFAULT, has_bias=False, reps=1,
                  skip_cc=False, phases=3, gather_mode="gather", proj=True):
    """Emit the SPMD bass program (identical on all cores)."""
    nc = bacc.Bacc("TRN2", target_bir_lowering=False, debug=False,
                   num_devices=cfg.NCORES, num_swdge_queues=4)
    P, BW, NBK, HID = cfg.P, cfg.BW, cfg.NBK, cfg.HID
    nT, tbs = layout["nT"], layout["tbs"]
    use_cc = cfg.NCORES > 1 and not skip_cc

    # ---------------- I/O (single packed input, single output) ----------
    OFF = pack_offsets(cfg, nT)
    pk_in = nc.dram_tensor("pk", [P, OFF["W"]], I16, kind="ExternalInput")

    def fsec(name, n, rows=P):
        o = OFF[name]
        return pk_in.ap()[:rows, o:o + 2 * n].bitcast(F32)

    def bsec(name, n, rows=P):
        o = OFF[name]
        return pk_in.ap()[:rows, o:o + n].bitcast(BF16)

    def isec(name, n):
        o = OFF[name]
        return pk_in.ap()[:, o:o + n]

    xT_b = pk_in.ap()[:, OFF["xT"]:OFF["xT"] + cfg.NPAD].bitcast(BF16)
    if has_bias:
        b1_in = nc.dram_tensor("b1", [1, HID], F32, kind="ExternalInput")
        bmu_in = nc.dram_tensor("bmu", [1, HID], F32, kind="ExternalInput")
        blv_in = nc.dram_tensor("blv", [1, HID], F32, kind="ExternalInput")
    z_out = nc.dram_tensor("z", [P, NBK // 2, 2 * HID], BF16,
                           kind="ExternalOutput")

    with tile.TileContext(nc) as tc:
        import contextlib
        stack = contextlib.ExitStack()
        with stack:
            dram = stack.enter_context(tc.tile_pool(name="dram", bufs=1, space="DRAM"))
            cpool = stack.enter_context(tc.tile_pool(name="const", bufs=1))

            us_tab = dram.tile([cfg.NPAD, 2 * HID], BF16)
            hs2_bnc = dram.tile([cfg.SHARD, HID], BF16)
            hs2_pk = dram.tile([cfg.NPAD, HID], BF16, addr_space="Shared")
            hs2_tab = dram.tile([cfg.NPAD, 2 * HID], BF16)

            w1_f = cpool.tile([cfg.IN, HID], F32)
            nc.sync.dma_start(out=w1_f[:], in_=fsec("w1", HID))
            w1_sb = cpool.tile([cfg.IN, HID], BF16)
            nc.vector.tensor_copy(out=w1_sb[:], in_=w1_f[:])
            wml_f = cpool.tile([HID, 2 * HID], F32)
            nc.sync.dma_start(out=wml_f[:], in_=fsec("wml", 2 * HID, HID))
            wml_sb = cpool.tile([HID, 2 * HID], BF16)
            nc.vector.tensor_copy(out=wml_sb[:], in_=wml_f[:])
            dis_own = cpool.tile([BW, NBK], F32)
            nc.sync.dma_start(out=dis_own[:], in_=fsec("dis", NBK, BW))
            disP = cpool.tile([P, NBK // 2], F32)
            nc.sync.dma_start(out=disP[:], in_=fsec("disP", NBK // 2))
            dis2 = cpool.tile([BW, NBK], F32)
            nc.vector.tensor_tensor(out=dis2[:], in0=dis_own[:],
                                    in1=dis_own[:], op=mybir.AluOpType.mult)

            iota_i = cpool.tile([P, BW], I32)
            nc.gpsimd.iota(iota_i[:], pattern=[[1, BW]], base=0,
                           channel_multiplier=0)
            iota_b = cpool.tile([P, BW], BF16)
            nc.vector.tensor_copy(out=iota_b[:], in_=iota_i[:])

            ident = cpool.tile([P, P], F32)
            make_identity(nc, ident[:])
            ident_bf = cpool.tile([P, P], BF16)
            nc.vector.tensor_copy(out=ident_bf[:], in_=ident[:])

            idx_sb = cpool.tile([P, nT * 8], I16, tag="idx", name="idx")
            nc.sync.dma_start(out=idx_sb[:], in_=isec("idx", nT * 8))
            dl_sb = cpool.tile([P, 2 * nT], BF16, tag="dl", name="dls")
            nc.sync.dma_start(out=dl_sb[:], in_=bsec("dl", 2 * nT))

            if has_bias:
                brow = cpool.tile([1, 3 * HID], F32)
                nc.sync.dma_start(out=brow[:, 0:HID], in_=b1_in.ap()[:])
                nc.sync.dma_start(out=brow[:, HID:2 * HID], in_=bmu_in.ap()[:])
                nc.sync.dma_start(out=brow[:, 2 * HID:], in_=blv_in.ap()[:])
                bias_bc = cpool.tile([P, 3 * HID], F32)
                nc.gpsimd.partition_broadcast(bias_bc[:], brow[:])

            def build_S(spool, tag):
                """One-hot S columns; col 2t = tile t's even rows, 2t+1 =
                odd rows (dead-marked dl kills the other parity)."""
                ncol = 2 * nT
                tiles = []
                for k0 in range(0, ncol, cfg.SB):
                    kn = min(cfg.SB, ncol - k0)
                    st = spool.tile([P, cfg.SB, BW], BF16, tag=tag,
                                    name=f"S{tag}")
                    nc.vector.tensor_tensor(
                        out=st[:, :kn, :],
                        in0=dl_sb[:, k0:k0 + kn].to_broadcast([P, kn, BW]),
                        in1=iota_b[:, None, :].to_broadcast([P, kn, BW]),
                        op=mybir.AluOpType.is_equal,
                    )
                    tiles.append(st)

                def one(k):
                    return tiles[k // cfg.SB][:, k % cfg.SB, :]

                return one

            qctr = [0]

            def gather_all(mpool, table, tagp):
                """Gather 512B row-PAIR elements chunk by chunk."""
                pair_ap = table.rearrange("(q two) f -> q (two f)", two=2)
                tiles = []
                for ci, t0 in enumerate(range(0, nT, cfg.CH)):
                    tn = min(cfg.CH, nT - t0)
                    mt = mpool.tile([P, cfg.CH, 4 * HID], BF16,
                                    tag=f"{tagp}{ci % 2}", name=f"M{tagp}")
                    nc.gpsimd.dma_gather(
                        out_ap=mt[:, :tn, :],
                        in_ap=pair_ap,
                        idxs_ap=idx_sb[:, t0 * 8:(t0 + tn) * 8],
                        num_idxs=tn * 128, num_idxs_reg=tn * 128,
                        elem_size=4 * HID,
                        single_packet=(tn * 128 <= 512),
                        queue_num=qctr[0] % 4,
                    )
                    qctr[0] += 1
                    tiles.append(mt)

                def msg(t, par):
                    return tiles[t // cfg.CH][:, t % cfg.CH,
                                              par * 2 * HID:
                                              par * 2 * HID + HID]

                return msg

            entries = [[] for _ in range(NBK)]
            for t, b in enumerate(tbs):
                entries[int(b)].append(t)

            for _rep in range(reps):
                # ========= PHASE A: u = x @ W1, scaled by dis -> us table ====
                it_stack = contextlib.ExitStack()
                with it_stack:
                    xa = it_stack.enter_context(tc.tile_pool(name="xa", bufs=2))
                    usb = it_stack.enter_context(tc.tile_pool(name="usb", bufs=2))
                    spool = it_stack.enter_context(tc.tile_pool(name="spool", bufs=2))
                    mpool = it_stack.enter_context(tc.tile_pool(name="mpool", bufs=MPOOL_BUFS))
                    hb = it_stack.enter_context(tc.tile_pool(name="hb", bufs=2))

                    XC = 16  # buckets per psum tile / ACT copy batch
                    # phase-A psum pool is scoped: all 8 banks are free here
                    XL = 49  # buckets per xT DMA (2 loads per shard)
                    if phases < 1:
                        zfill0 = usb.tile([BW, NBK, HID], BF16, tag="usbb",
                                          name="zfill0")
                        nc.vector.memset(zfill0[:], 0)
                    c2_order = [0, 1, 2, 3, 4, 5, 6, 7][:cfg.NCORES]
                    a_stack = contextlib.ExitStack()
                    pu = a_stack.enter_context(
                        tc.tile_pool(name="pu", bufs=4, space="PSUM"))
                    for c2 in (c2_order if phases >= 1 else []):
                        us_blk = usb.tile([BW, NBK, HID], BF16, tag="usbb",
                                          name="us_blk")
                        xtbs = {}
                        for L0 in range(0, NBK, XL):
                            ln = min(XL, NBK - L0)
                            xtb = xa.tile([P, XL, BW], BF16, tag="xtb",
                                          name="xtb")
                            nc.sync.dma_start(
                                out=xtb[:, :ln, :],
                                in_=xT_b[:, c2 * cfg.SHARD + L0 * BW:
                                         c2 * cfg.SHARD + (L0 + ln) * BW]
                                .rearrange("p (t q) -> p t q", t=ln))
                            xtbs[L0] = xtb
                        for B0 in range(0, NBK, XC):
                            bn = min(XC, NBK - B0)
                            ups = pu.tile([BW, XC, HID], F32, space="PSUM",
                                          tag="u", name="ups")
                            for j in range(bn):
                                L0 = ((B0 + j) // XL) * XL
                                nc.tensor.matmul(out=ups[:, j, :],
                                                 lhsT=xtbs[L0][:, B0 + j - L0, :],
                                                 rhs=w1_sb[:],
                                                 start=True, stop=True)
                            # dis[src] is folded into x on the host, so this
                            # is a pure psum->sbuf bf16 cast (ACT engine).
                            nc.scalar.copy(out=us_blk[:, B0:B0 + bn, :],
                                           in_=ups[:, :bn, :])
                        nc.sync.dma_start(
                            out=us_tab[c2 * cfg.SHARD:(c2 + 1) * cfg.SHARD,
                                       0:HID]
                            .rearrange("(j b) f -> j b f", j=BW),
                            in_=us_blk[:])
                    a_stack.close()
                    pagg = it_stack.enter_context(
                        tc.tile_pool(name="pagg", bufs=3, space="PSUM"))
                    pproj = it_stack.enter_context(
                        tc.tile_pool(name="pproj", bufs=2, space="PSUM"))
                    pz = it_stack.enter_context(
                        tc.tile_pool(name="pz", bufs=1, space="PSUM"))

                    # ================= PHASE B: layer-1 aggregation =============
                    if phases >= 2:
                        msg = gather_all(mpool, us_tab[:], "m")
                        S1 = build_S(spool, "s1")
                        hs2_sb = usb.tile([BW, NBK, HID], BF16, tag="usbb",
                                          name="hs2_sb")
                        for b0 in range(0, NBK, 2):
                            ps = pagg.tile([BW, 2, HID], F32, space="PSUM",
                                           tag="agg", name="ps1")
                            for k in (0, 1):
                                ent = entries[b0 + k]
                                for i, t in enumerate(ent):
                                    for par in (0, 1):
                                        nc.tensor.matmul(
                                            out=ps[:, k, :],
                                            lhsT=S1(2 * t + par),
                                            rhs=msg(t, par),
                                            start=(i == 0 and par == 0),
                                            stop=(i == len(ent) - 1
                                                  and par == 1))
                            if has_bias:
                                t1 = hb.tile([BW, 2, HID], F32, tag="h",
                                             name="t1")
                                # h1 = relu(ps*dis + b); hs2 = h1*dis
                                dpair = dis_own[:, b0:b0 + 2, None] \
                                    .to_broadcast([BW, 2, HID])
                                nc.vector.tensor_tensor(
                                    out=t1[:], in0=ps[:], in1=dpair,
                                    op=mybir.AluOpType.mult)
                                nc.vector.tensor_tensor(
                                    out=t1[:], in0=t1[:],
                                    in1=bias_bc[:BW, None, 0:HID]
                                    .to_broadcast([BW, 2, HID]),
                                    op=mybir.AluOpType.add)
                                nc.vector.tensor_relu(out=t1[:], in_=t1[:])
                                nc.vector.tensor_tensor(
                                    out=hs2_sb[:, b0:b0 + 2, :], in0=t1[:],
                                    in1=dpair, op=mybir.AluOpType.mult)
                            else:
                                # dis>0: relu(ps)*dis^2 == relu(ps*dis^2);
                                # one ACT op per bucket keeps DVE free for
                                # the S-matrix builds.
                                for k in (0, 1):
                                    nc.scalar.activation(
                                        out=hs2_sb[:, b0 + k, :],
                                        in_=ps[:, k, :],
                                        func=mybir.ActivationFunctionType.Relu,
                                        scale=dis2[:, b0 + k, None])
                        if use_cc:
                            nc.sync.dma_start(
                                out=hs2_bnc[:].rearrange("(j b) f -> j b f", j=BW),
                                in_=hs2_sb[:])
                            nc.gpsimd.collective_compute(
                                "AllGather", mybir.AluOpType.bypass,
                                replica_groups=[list(range(cfg.NCORES))],
                                ins=[hs2_bnc.opt()], outs=[hs2_pk.opt()],
                            )
                            nc.sync.dma_start(out=hs2_tab[:, 0:HID],
                                              in_=hs2_pk[:])
                        else:
                            nc.sync.dma_start(
                                out=hs2_pk[:cfg.SHARD, :]
                                .rearrange("(j b) f -> j b f", j=BW),
                                in_=hs2_sb[:])
                            nc.sync.dma_start(out=hs2_tab[:, 0:HID],
                                              in_=hs2_pk[:])

                    # ============== PHASE C: layer-2 + projections ==============
                    if phases >= 3:
                        msg = gather_all(mpool, hs2_tab[:], "m")
                        S2 = build_S(spool, "s1")
                        # feature-major aggregation: aggT[f, node]
                        a2T_sb = usb.tile([HID, NBK * BW], BF16, tag="usb",
                                          name="a2T_sb")
                        for b0 in range(0, NBK, 2):
                            ps = pagg.tile([HID, 2, BW], F32, space="PSUM",
                                           tag="agg", name="ps2")
                            for k in (0, 1):
                                ent = entries[b0 + k]
                                for i, t in enumerate(ent):
                                    for par in (0, 1):
                                        nc.tensor.matmul(
                                            out=ps[:, k, :],
                                            lhsT=msg(t, par),
                                            rhs=S2(2 * t + par),
                                            start=(i == 0 and par == 0),
                                            stop=(i == len(ent) - 1
                                                  and par == 1))
                            nc.scalar.copy(
                                out=a2T_sb[:, b0 * BW:(b0 + 2) * BW],
                                in_=ps[:])
                        if not proj:
                            nc.sync.dma_start(
                                out=z_out.ap()[0:HID, :, :],
                                in_=a2T_sb[:].rearrange(
                                    "p (t q) -> p t q", q=2 * HID))
                        # zcatT = [Wmu | Wlv].T @ aggT  -> [2H, nodes]
                        zT_sb = usb.tile([2 * HID, NBK * BW], BF16, tag="usb",
                                         name="zT_sb")
                        CHK = 512
                        for n0 in (range(0, NBK * BW, CHK) if proj else []):
                            cn = min(CHK, NBK * BW - n0)
                            zT_ps = pproj.tile([2 * HID, CHK], F32,
                                               space="PSUM", tag="zT",
                                               name="zT_ps")
                            nc.tensor.matmul(out=zT_ps[:, :cn],
                                             lhsT=wml_sb[:],
                                             rhs=a2T_sb[:, n0:n0 + cn],
                                             start=True, stop=True)
                            nc.scalar.copy(out=zT_sb[:, n0:n0 + cn],
                                           in_=zT_ps[:, :cn])
                        # transpose back per 128-node pair, scale by dis
                        zcat_sb = usb.tile([P, NBK // 2, 2 * HID], BF16,
                                           tag="usb", name="zcat_sb")
                        for t in (range(NBK // 2) if proj else []):
                            z_ps = pz.tile([P, P], BF16, space="PSUM",
                                           tag="z", name="z_ps")
                            nc.tensor.transpose(
                                out=z_ps[:],
                                in_=zT_sb[:, t * P:(t + 1) * P],
                                identity=ident_bf[:])
                            nc.scalar.mul(out=zcat_sb[:, t, :], in_=z_ps[:],
                                          mul=disP[:, t, None])
                        if has_bias and proj:
                            nc.vector.tensor_tensor(
                                out=zcat_sb[:], in0=zcat_sb[:],
                                in1=bias_bc[:, None, HID:3 * HID]
                                .to_broadcast([P, NBK // 2, 2 * HID]),
                                op=mybir.AluOpType.add)
                        if proj:
                            nc.sync.dma_start(out=z_out.ap()[:],
                                              in_=zcat_sb[:])
                    if phases < 3:
                        zfill = usb.tile([P, NBK // 2, 2 * HID], BF16,
                                         tag="usb", name="zfill")
                        nc.vector.memset(zfill[:], 0)
                        nc.sync.dma_start(out=z_out.ap()[:], in_=zfill[:])

    nc.compile()
    return nc


_CACHE = {}


def _get_program(edge_index, cfg, has_bias):
    layout = build_layout(edge_index, cfg)
    key = (layout["ntile"], has_bias)
    if key not in _CACHE:
        _CACHE[key] = build_program(layout, cfg, has_bias)
    return _CACHE[key], layout


def make_in_maps(x, edge_index, W1, b1, Wmu, bmu, Wlv, blv, layout,
                 cfg=DEFAULT, has_bias=False):
    x = np.asarray(x, np.float32)
    xpad = np.zeros((cfg.NPAD, cfg.IN), np.float32)
    xpad[:x.shape[0]] = x
    # fold dis[src] into x so phase A's matmul directly yields us = dis*(x@W1)
    xpad *= layout["dis_full"][:, None]
    xT = np.ascontiguousarray(xpad.T)
    wml = np.concatenate([np.asarray(Wmu, np.float32),
                          np.asarray(Wlv, np.float32)], axis=1)
    w1 = np.asarray(W1, np.float32)
    nT = layout["nT"]
    OFF = pack_offsets(cfg, nT)

    def put_f32(pk, name, arr):
        arr = np.asarray(arr, np.float32)
        o = OFF[name]
        pk[:arr.shape[0], o:o + 2 * arr.shape[1]] = arr.view(np.int16)

    def put_bf16(pk, name, arr):
        import ml_dtypes
        arr = np.asarray(arr, np.float32).astype(ml_dtypes.bfloat16)
        o = OFF[name]
        pk[:arr.shape[0], o:o + arr.shape[1]] = arr.view(np.int16)

    maps = []
    for c in range(cfg.NCORES):
        pk = np.zeros((cfg.P, OFF["W"]), np.int16)
        cd = layout["cores"][c]
        put_bf16(pk, "xT", xT)
        put_f32(pk, "w1", w1)
        put_f32(pk, "wml", wml)
        put_f32(pk, "dis", cd["dis"])
        put_f32(pk, "disP", cd["disP"])
        put_bf16(pk, "dl", cd["dl"])
        pk[:, OFF["idx"]:OFF["idx"] + 8 * nT] = cd["idx"]
        m = dict(pk=pk)
        if has_bias:
            m.update(b1=np.asarray(b1, np.float32).reshape(1, -1),
                     bmu=np.asarray(bmu, np.float32).reshape(1, -1),
                     blv=np.asarray(blv, np.float32).reshape(1, -1))
        maps.append(m)
    return maps


def unshard(results, cfg=DEFAULT):
    H = cfg.HID
    zmu_blocks, zlv_blocks = [], []
    for c in range(cfg.NCORES):
        z = np.asarray(results[c]["z"]).astype(np.float32)
        z4 = z.reshape(2, cfg.BW, cfg.NBK // 2, 2 * H)
        zjb = np.transpose(z4, (2, 0, 1, 3)).reshape(cfg.NBK, cfg.BW, 2 * H)
        # zjb[b, j, :]: node c*SHARD + b*BW + j
        zmu_blocks.append(zjb[:, :, 0:H].reshape(cfg.SHARD, H))
        zlv_blocks.append(zjb[:, :, H:2 * H].reshape(cfg.SHARD, H))
    return (np.concatenate(zmu_blocks, axis=0)[:cfg.N],
            np.concatenate(zlv_blocks, axis=0)[:cfg.N])


def kernel(x, edge_index, W1, b1, Wmu, bmu, Wlv, blv):
    cfg = DEFAULT
    has_bias = any(np.any(np.asarray(b)) for b in (b1, bmu, blv))
    nc, layout = _get_program(np.asarray(edge_index), cfg, has_bias)
    in_maps = make_in_maps(x, edge_index, W1, b1, Wmu, bmu, Wlv, blv,
                           layout, cfg, has_bias)
    res = run_bass_kernel_spmd(nc, in_maps, core_ids=list(range(cfg.NCORES)))
    return unshard(res.results, cfg)

